# revision 27
# baseline (speedup 1.0000x reference)
"""Expert-parallel MoE "behind" block + residual on 8 Trainium2 NeuronCores.

Reference computation (fp32):
    front      = inputs[:E*C].reshape(E, C, D_IN)
    expert_out = einsum("ecd,edm->ecm", front, expert_w) + expert_b
    combined   = einsum("sec,ecm->sm", combine_weights, expert_out)
    resid      = inputs[E*C:] @ residual_w + residual_b
    out        = combined * w0[:, None] + resid * w1[:, None]

Sharding (8 cores):
  Stage 1 (expert-parallel): core e computes eo_e = front_e @ W_e  [C, D_OUT],
  in two c-halves; each half is AllGathered over the cores as soon as it is
  ready (2 chunked AllGathers overlap stage-1/3 compute on the PE).
  Stage 3 (token-parallel residual): core r owns tokens S_r (512 rows) and
  accumulates (w1*resid)[S_r] @ residual_w into 8 PSUM banks.
  Stage 2 (token-parallel combine): accumulates (w0*cw)[S_r] @ eo_full into
  the same PSUM banks (w0/w1 folded into cw / resid rows on the host; exact).
  The (all-zero) bias terms are added back exactly on the host:
      out += w1 x residual_b  +  w0 * (cw.sum(c) @ expert_b)

All device matmuls contract over the SBUF partition axis, so every DRAM
operand is laid out contraction-major on the host.  The chunked AllGather
concatenates per-rank c-halves, so cwT's contraction rows are ordered
(chunk, expert, c-within-half) to match.

Modes (env TRN_KERNEL_MODE): "bf16" (default) ships bf16 operands with fp32
PSUM accumulate.  "fp32" is the exact fallback (plain fp32 PE at 4
cycles/row) — 1.37 ms, rel-l2 6.6e-7.  "fp32r" compiles but mis-computes on
this hardware (15% error) — do not use.

Stage 3 runs in fp8-e4m3 DoubleRow perf mode (TRN_S3FP8=1, default): each
matmul contracts two 128-k tiles at 0.5 PE cycles per output element,
halving that stage's PE-active time (67 -> 34 us).  The residual term is
~50x smaller than the combine term, so fp8 there costs only ~1.5e-3 of
rel-l2 (3.3e-3 -> 4.8e-3, gate is 2e-2).  Stages 1/2 must stay bf16: fp8 on
either combine operand or the expert matmul measures 2.4e-2..5.6e-2.

Perf notes (measured on these cores): the PE is power-throttled
(throttle_gpio_2 active ~91%, util limit ~78%) — sustained bf16 cadence is
262 ns per 512-wide matmul vs the 213 ns unthrottled streaming time, with
LDWEIGHTS already hidden by hardware double-buffering.  The kernel is
therefore PE-cycle-bound, not schedule-bound; bf16 561 us -> with fp8
stage 3 measured ~329-340 us (run-to-run spread ±6 us from the throttle
and the cross-core start barrier).  Stage-1's t=40..130 us window runs at
~280-335 GB/s of the 358 GB/s DMA cap (ft/we feed + AllGather-0 + stage-3
full-residency prefetch) — moving the prefetch anywhere else measured
worse; see the inline comments.
"""

import os
import numpy as np
import ml_dtypes

E, C, D_IN, D_OUT = 8, 1024, 4096, 1024
B, S = 2, 2048
TOK = B * S                 # 4096 tokens
N_CORES = 8
S_LOC = TOK // N_CORES      # 512 tokens per core
CH = C // 2                 # c-half = 512
BF16 = ml_dtypes.bfloat16

MODE = os.environ.get("TRN_KERNEL_MODE", "bf16")
LDW_OPT = os.environ.get("TRN_LDW_OPT", "0") == "1"
SKIP_LDW = os.environ.get("TRN_SKIP_LDW", "1") == "1"
NO_MOVE_WAITS = os.environ.get("TRN_NO_MOVE_WAITS", "0") == "1"
# stage-3 residual matmul in fp8 DoubleRow (2 k-tiles/instr, 2x PE rate).
# The residual contribution is ~50x smaller than the combine term, so fp8
# quantization there is invisible (simulated rel-l2 3.35e-3 vs 3.25e-3).
S3FP8 = os.environ.get("TRN_S3FP8", "1") == "1"
F8 = ml_dtypes.float8_e4m3fn

_prog_cache = {}


def _patch_ldw_opt():
    """Compile this kernel's NEFF with walrus' LDWEIGHTS double-buffering
    (--enable-ldw-opt=true): hides the per-matmul 128-column weight load
    behind the previous matmul (~50 ns/MM here). Wrapped so only our
    compile is affected."""
    from concourse import bass_utils
    if getattr(bass_utils, "_ldw_opt_patched", False):
        return
    orig = bass_utils.run_command

    def patched(argv, **kw):
        argv = ["--enable-ldw-opt=true" if a == "--enable-ldw-opt=false" else a
                for a in argv]
        return orig(argv, **kw)

    bass_utils.run_command = patched
    bass_utils._ldw_opt_patched = True


def _build(mode, ldw_opt):
    import concourse.bass as bass  # noqa: F401
    import concourse.mybir as mybir
    from concourse import bacc
    from concourse.tile import TileContext, add_dep_helper

    dt = mybir.dt
    # fp32r must be the declared dtype end-to-end (the BIR verifier rejects
    # fp32-typed producers feeding fp32r matmuls), not a bitcast at the matmul
    io_dt = {"bf16": dt.bfloat16, "fp32r": dt.float32r, "fp32": dt.float32}[mode]
    mm_cast = lambda ap: ap

    nc = bacc.Bacc("TRN2", target_bir_lowering=False, debug=False, num_devices=N_CORES)
    if NO_MOVE_WAITS:
        # keep semaphore waits on the matmuls (not the ldweights) so walrus'
        # --enable-ldw-opt pass accepts the program; instance-level no-op
        nc.move_matmul_waits_to_ldweights = lambda: None

    s3_dt = dt.float8e4 if S3FP8 else io_dt
    fT = nc.declare_dram_parameter("fT", [D_IN, C], io_dt, isOutput=False)
    we = nc.declare_dram_parameter("we", [D_IN, D_OUT], io_dt, isOutput=False)
    cwT = nc.declare_dram_parameter("cwT", [E * C, S_LOC], io_dt, isOutput=False)
    riT = nc.declare_dram_parameter("riT", [D_IN, S_LOC], s3_dt, isOutput=False)
    rw = nc.declare_dram_parameter("rw", [D_IN, D_OUT], s3_dt, isOutput=False)
    out = nc.declare_dram_parameter("out", [S_LOC, D_OUT], dt.float32, isOutput=True)

    # variant tag in a tensor name so differently-compiled builds never share
    # a jax compile-cache entry
    nc.dram_tensor(f"variant_{mode}_{int(ldw_opt)}_{int(SKIP_LDW)}_{int(NO_MOVE_WAITS)}"
                   f"_{int(S3FP8)}", [1, 1], dt.float32)

    ag_in = [nc.dram_tensor(f"ag_in{h}", [CH, D_OUT], io_dt) for h in range(2)]
    ag_out = [nc.dram_tensor(f"ag_out{h}", [N_CORES * CH, D_OUT], io_dt,
                             addr_space="Shared") for h in range(2)]

    KT = D_IN // 128            # 32 contraction tiles
    SUB = 4                     # k-subtiles per DMA'd block
    NBLK = KT // SUB            # 8 blocks
    ECT = (E * C) // 128        # 64 combine contraction tiles
    NFREE = 512                 # ISA cap: s3d3_mm_num_elements <= 512
    NJ = D_OUT // NFREE
    rearr = lambda a: a.rearrange("(n p) d -> p n d", p=128)

    S1SUB = 2                   # finer stage-1 blocks: deeper prefetch pipeline
    S1BLK = KT // S1SUB         # 16 blocks

    bf16_mode = io_dt == dt.bfloat16
    B_FT, B_WE, B_RI, B_RW, B_CW, B_EOAG, B_EO = (
        (12, 9, 3, 3, 3, 6, 2) if bf16_mode else (4, 4, 2, 2, 2, 2, 1))
    if S3FP8 and bf16_mode:
        # fp8 ri/rw tiles are half-size; make them fully resident (8 blocks
        # each) so stage 3 runs with zero DMA waits — its loads prefetch
        # during stage 1, before AllGather chunk 1 saturates the DMA rings
        # (measured 9 us stage-3 stall with 3 bufs).
        B_FT, B_RI, B_RW = 12, 8, 8
    with TileContext(nc) as tc:
        with tc.tile_pool(name="p_ft", bufs=B_FT) as p_ft, \
             tc.tile_pool(name="p_we", bufs=B_WE) as p_we, \
             tc.tile_pool(name="p_ri", bufs=B_RI) as p_ri, \
             tc.tile_pool(name="p_rw", bufs=B_RW) as p_rw, \
             tc.tile_pool(name="p_cw", bufs=B_CW) as p_cw, \
             tc.tile_pool(name="p_eoag", bufs=B_EOAG) as p_eoag, \
             tc.tile_pool(name="p_eo", bufs=B_EO) as p_eo, \
             tc.tile_pool(name="p_out", bufs=1) as p_out, \
             tc.tile_pool(name="psum", bufs=1, space="PSUM") as p_ps:


            def mm_pair(psrow, lhsT_ap, rhs_of_j, start, stop):
                """Two matmuls sharing one stationary operand: the second
                skips its LDWEIGHTS (identical weights already in the array)
                and is order-pinned right after the first."""
                prev = None
                for j in range(NJ):
                    m = nc.tensor.matmul(psrow[j], lhsT_ap, rhs_of_j(j),
                                         start=start, stop=stop)
                    # fp32's two-pass matmul requires self-loading weights
                    if j > 0 and SKIP_LDW and bf16_mode:
                        m.ins.ldweights = False
                        add_dep_helper(m.ins, prev.ins, False, "weight-reuse pair order")
                    prev = m
                return prev

            def psum_tiles(tagp):
                return [[p_ps.tile([128, NFREE], dt.float32,
                                   name=f"{tagp}_{i}_{j}", tag=f"ps_{i}_{j}")
                         for j in range(NJ)] for i in range(4)]

            # Stage-3 fp8 operands are fully resident (8 bufs each): their
            # dma_starts sit at the stage-3 program position on sync/scalar,
            # which self-paces them in queue-FIFO order behind all stage-1
            # ft/we loads.  (Every attempt to move or pace them differently —
            # SWDGE burst, SWDGE paced, FIFO-interleaved with stage-1 loads —
            # measured 6-10 us WORSE by starving stage-1's just-in-time feed.)
            rearr2 = lambda a: a.rearrange("(s t p) d -> p s t d", p=128, t=2)

            # ------------- Stage 1: eo_e = fT.T @ we, by c-halves ------------
            last_we_dma = None
            for ch in range(2):
                c0 = ch * CH
                psums = psum_tiles(f"s1h{ch}")
                for blk in range(S1BLK):
                    r0 = blk * 128 * S1SUB
                    # only this c-half's columns of fT are needed
                    ft_t = p_ft.tile([128, S1SUB, CH], io_dt, tag="ft", name=f"ft_{ch}_{blk}")
                    we_t = p_we.tile([128, S1SUB, D_OUT], io_dt, tag="we", name=f"we_{ch}_{blk}")
                    if ch == 0 and blk == 0:
                        # split the very first loads finer still: the first
                        # mm_pair (i=0, j=0) starts after ft cols 0:256
                        # (64 KiB) + we cols 0:512 (128 KiB) land, not after
                        # the full 384 KiB sub — worth ~1.5 us at kernel start
                        for sub in range(S1SUB):
                            for ih in range(2):
                                nc.sync.dma_start(
                                    out=ft_t[:, sub:sub + 1, ih * 256:(ih + 1) * 256],
                                    in_=rearr(fT[r0 + sub * 128:r0 + (sub + 1) * 128,
                                                 c0 + ih * 256:c0 + (ih + 1) * 256]))
                            for jh in range(2):
                                last_we_dma = nc.scalar.dma_start(
                                    out=we_t[:, sub:sub + 1, jh * 512:(jh + 1) * 512],
                                    in_=rearr(we[r0 + sub * 128:r0 + (sub + 1) * 128,
                                                 jh * 512:(jh + 1) * 512]))
                    else:
                        nc.sync.dma_start(
                            out=ft_t, in_=rearr(fT[r0:r0 + 128 * S1SUB, c0:c0 + CH]))
                        # scalar queue: second HWDGE ring, parallel with sync's
                        last_we_dma = nc.scalar.dma_start(
                            out=we_t, in_=rearr(we[r0:r0 + 128 * S1SUB, :]))
                    for sub in range(S1SUB):
                        kt = blk * S1SUB + sub
                        for i in range(4):
                            mm_pair(psums[i],
                                    mm_cast(ft_t[:, sub, i * 128:(i + 1) * 128]),
                                    lambda j, sub=sub: mm_cast(we_t[:, sub, j * NFREE:(j + 1) * NFREE]),
                                    start=(kt == 0), stop=(kt == KT - 1))

                eo_half = p_eo.tile([128, 4, D_OUT], io_dt, tag="eo", name=f"eo_{ch}")
                for i in range(4):
                    for j in range(NJ):
                        nc.vector.tensor_copy(out=eo_half[:, i, j * NFREE:(j + 1) * NFREE],
                                              in_=psums[i][j])
                # gpsimd (SWDGE) queue: keeps this late-gated write out of the
                # HWDGE FIFOs so it can't head-of-line block operand loads
                nc.gpsimd.dma_start(out=rearr(ag_in[ch][:]), in_=eo_half)
                # chunked AllGather: starts while the PE grinds the next phase
                nc.gpsimd.collective_compute(
                    "AllGather", mybir.AluOpType.bypass,
                    replica_groups=[list(range(N_CORES))],
                    ins=[ag_in[ch][:].opt()], outs=[ag_out[ch][:].opt()])

            # ------------- Stage 3: resid partial (w1 folded) ----------------
            psums = psum_tiles("s23")
            last_rw_dma = None
            if S3FP8:
                # fp8 DoubleRow: each matmul contracts TWO 128-k tiles
                # (lhsT [128,2,128], rhs [128,2,256] -> psum [128,256]),
                # halving the PE-active cycles of this stage.
                for blk in range(NBLK):
                    ri_t = p_ri.tile([128, 2, 2, S_LOC], s3_dt, tag="ri",
                                     name=f"ri_{blk}")
                    nc.sync.dma_start(
                        out=ri_t, in_=rearr2(riT[blk * 512:(blk + 1) * 512, :]))
                    rw_t = p_rw.tile([128, 2, 2, D_OUT], s3_dt, tag="rw",
                                     name=f"rw_{blk}")
                    nc.scalar.dma_start(
                        out=rw_t, in_=rearr2(rw[blk * 512:(blk + 1) * 512, :]))
                    for sub in range(2):
                        first = blk == 0 and sub == 0
                        for i in range(4):
                            for n in range(4):
                                nc.tensor.matmul(
                                    psums[i][n // 2][:, (n % 2) * 256:(n % 2) * 256 + 256],
                                    ri_t[:, sub, :, i * 128:(i + 1) * 128],
                                    rw_t[:, sub, :, n * 256:(n + 1) * 256],
                                    start=first, stop=False,
                                    perf_mode=mybir.MatmulPerfMode.DoubleRow,
                                    skip_group_check=True)
            else:
                for blk in range(NBLK):
                    ri_t = p_ri.tile([128, SUB, S_LOC], io_dt, tag="ri", name=f"ri_{blk}")
                    nc.sync.dma_start(out=ri_t, in_=rearr(riT[blk * 512:(blk + 1) * 512, :]))
                    rw_t = p_rw.tile([128, SUB, D_OUT], io_dt, tag="rw", name=f"rw_{blk}")
                    last_rw_dma = nc.scalar.dma_start(
                        out=rw_t, in_=rearr(rw[blk * 512:(blk + 1) * 512, :]))
                    for sub in range(SUB):
                        kt = blk * SUB + sub
                        for i in range(4):
                            mm_pair(psums[i],
                                    mm_cast(ri_t[:, sub, i * 128:(i + 1) * 128]),
                                    lambda j, sub=sub: mm_cast(rw_t[:, sub, j * NFREE:(j + 1) * NFREE]),
                                    start=(kt == 0), stop=False)

            # ------------- Stage 2: combine partial (w0 folded) --------------
            out_sb = p_out.tile([128, 4, D_OUT], dt.float32)
            prev_eoag_dma = None
            for blk in range(ECT // SUB):
                half = blk // 8              # ag chunk this block reads
                r0 = (blk % 8) * 512
                cw_t = p_cw.tile([128, SUB, S_LOC], io_dt, tag="cw", name=f"cw_{blk}")
                nc.sync.dma_start(out=cw_t, in_=rearr(cwT[blk * 512:(blk + 1) * 512, :]))
                eo_t = p_eoag.tile([128, SUB, D_OUT], io_dt, tag="eoag", name=f"eoag_{blk}")
                eoag_dma = nc.scalar.dma_start(out=eo_t, in_=rearr(ag_out[half][r0:r0 + 512, :]))
                # Scalar-queue order: eoag loads wait on the AllGathers, so pin
                # them after every stage-1/3 operand load and in block order —
                # otherwise the scheduler can hoist one ahead and head-of-line
                # block the HWDGE FIFO on the collective (measured 44 us stall).
                prev = prev_eoag_dma if prev_eoag_dma is not None else (
                    last_rw_dma if last_rw_dma is not None else last_we_dma)
                if prev is not None:
                    add_dep_helper(eoag_dma.ins, prev.ins, False,
                                   "eoag after stage-1/3 loads, in block order")
                prev_eoag_dma = eoag_dma
                last_blk = blk == ECT // SUB - 1
                if not last_blk:
                    for sub in range(SUB):
                        for i in range(4):
                            mm_pair(psums[i],
                                    mm_cast(cw_t[:, sub, i * 128:(i + 1) * 128]),
                                    lambda j, sub=sub: mm_cast(eo_t[:, sub, j * NFREE:(j + 1) * NFREE]),
                                    start=False, stop=False)
                else:
                    # last block: finish groups one at a time so the PSUM->SBUF
                    # copies and output DMAs overlap the remaining matmuls
                    for i in range(4):
                        for j in range(NJ):
                            for sub in range(SUB):
                                nc.tensor.matmul(
                                    psums[i][j],
                                    mm_cast(cw_t[:, sub, i * 128:(i + 1) * 128]),
                                    mm_cast(eo_t[:, sub, j * NFREE:(j + 1) * NFREE]),
                                    start=False, stop=(sub == SUB - 1))
                            nc.vector.tensor_copy(
                                out=out_sb[:, i, j * NFREE:(j + 1) * NFREE],
                                in_=psums[i][j])
                        nc.sync.dma_start(
                            out=out[i * 128:(i + 1) * 128, :].rearrange("(n p) d -> p n d", p=128),
                            in_=out_sb[:, i:i + 1, :])

    nc.finalize()
    return nc


def _get_prog(mode, ldw_opt):
    key = (mode, ldw_opt)
    if key not in _prog_cache:
        if ldw_opt:
            _patch_ldw_opt()
        _prog_cache[key] = _build(mode, ldw_opt)
    return _prog_cache[key]


def _prep_in_maps(inputs, expert_w, residual_w, combine_weights, residual_weight, mode):
    np_dt = BF16 if mode == "bf16" else np.float32
    s3_dt = F8 if S3FP8 else np_dt
    front = inputs[:E * C].reshape(E, C, D_IN)
    resid = inputs[E * C:]                       # [TOK, D_IN]
    rwt = residual_weight.reshape(TOK, 2)
    w0, w1 = rwt[:, 0], rwt[:, 1]

    rw_cast = np.ascontiguousarray(residual_w.astype(s3_dt))
    resid_s = resid * w1[:, None]                # fold w1 (fp32)
    in_maps = []
    for r in range(N_CORES):
        sl = slice(r * S_LOC, (r + 1) * S_LOC)
        fT = np.ascontiguousarray(front[r].T.astype(np_dt))              # [D_IN, C]
        we = np.ascontiguousarray(expert_w[r].astype(np_dt))             # [D_IN, D_OUT]
        cw_s = combine_weights[sl] * w0[sl, None, None]                  # [S_LOC, E, C]
        # contraction rows ordered (c-half chunk, expert, c-within-half) to
        # match the chunked AllGather's concatenation
        cwT = np.ascontiguousarray(
            cw_s.reshape(S_LOC, E, 2, CH).transpose(2, 1, 3, 0).reshape(E * C, S_LOC)
            .astype(np_dt))
        riT = np.ascontiguousarray(resid_s[sl].T.astype(s3_dt))          # [D_IN, S_LOC]
        in_maps.append({"fT": fT, "we": we, "cwT": cwT, "riT": riT, "rw": rw_cast})
    return in_maps


def _run(inputs, expert_w, expert_b, residual_w, residual_b,
         combine_weights, residual_weight, mode=None, ldw_opt=None, trace=False):
    import jax
    try:
        if jax.config.jax_compilation_cache_dir is None:
            jax.config.update("jax_compilation_cache_dir", "/tmp/jax_cache_trn_moe")
            jax.config.update("jax_persistent_cache_min_compile_time_secs", 0.5)
    except Exception:
        pass
    from concourse.bass_utils import run_bass_kernel_spmd

    mode = mode or MODE
    ldw_opt = LDW_OPT if ldw_opt is None else ldw_opt
    inputs = np.asarray(inputs, dtype=np.float32)
    expert_w = np.asarray(expert_w, dtype=np.float32)
    expert_b = np.asarray(expert_b, dtype=np.float32)
    residual_w = np.asarray(residual_w, dtype=np.float32)
    residual_b = np.asarray(residual_b, dtype=np.float32)
    combine_weights = np.asarray(combine_weights, dtype=np.float32)
    residual_weight = np.asarray(residual_weight, dtype=np.float32)

    nc = _get_prog(mode, ldw_opt)
    in_maps = _prep_in_maps(inputs, expert_w, residual_w, combine_weights,
                            residual_weight, mode)
    res = run_bass_kernel_spmd(nc, in_maps, list(range(N_CORES)), trace=trace)
    out = np.concatenate([res.results[r]["out"] for r in range(N_CORES)], axis=0)

    # exact bias contributions (zero in practice, but keep the math honest)
    rwt = residual_weight.reshape(TOK, 2)
    if residual_b.any():
        out = out + rwt[:, 1:2] * residual_b[None, :]
    if expert_b.any():
        cs = combine_weights.sum(axis=2)                    # [TOK, E]
        out = out + rwt[:, 0:1] * (cs @ expert_b)
    return out.reshape(B, S, D_OUT).astype(np.float32), res


def kernel(**kw):
    out, _ = _run(**kw)
    return out



# revision 28
# speedup vs baseline: 1.0022x; 1.0022x over previous
"""Expert-parallel MoE "behind" block + residual on 8 Trainium2 NeuronCores.

Reference computation (fp32):
    front      = inputs[:E*C].reshape(E, C, D_IN)
    expert_out = einsum("ecd,edm->ecm", front, expert_w) + expert_b
    combined   = einsum("sec,ecm->sm", combine_weights, expert_out)
    resid      = inputs[E*C:] @ residual_w + residual_b
    out        = combined * w0[:, None] + resid * w1[:, None]

Sharding (8 cores):
  Stage 1 (expert-parallel): core e computes eo_e = front_e @ W_e  [C, D_OUT],
  in two c-halves; each half is AllGathered over the cores as soon as it is
  ready (2 chunked AllGathers overlap stage-1/3 compute on the PE).
  Stage 3 (token-parallel residual): core r owns tokens S_r (512 rows) and
  accumulates (w1*resid)[S_r] @ residual_w into 8 PSUM banks.
  Stage 2 (token-parallel combine): accumulates (w0*cw)[S_r] @ eo_full into
  the same PSUM banks (w0/w1 folded into cw / resid rows on the host; exact).
  The (all-zero) bias terms are added back exactly on the host:
      out += w1 x residual_b  +  w0 * (cw.sum(c) @ expert_b)

All device matmuls contract over the SBUF partition axis, so every DRAM
operand is laid out contraction-major on the host.  The chunked AllGather
concatenates per-rank c-halves, so cwT's contraction rows are ordered
(chunk, expert, c-within-half) to match.

Modes (env TRN_KERNEL_MODE): "bf16" (default) ships bf16 operands with fp32
PSUM accumulate.  "fp32" is the exact fallback (plain fp32 PE at 4
cycles/row) — 1.37 ms, rel-l2 6.6e-7.  "fp32r" compiles but mis-computes on
this hardware (15% error) — do not use.

Stage 3 runs in fp8-e4m3 DoubleRow perf mode (TRN_S3FP8=1, default): each
matmul contracts two 128-k tiles at 0.5 PE cycles per output element,
halving that stage's PE-active time (67 -> 34 us).  The residual term is
~50x smaller than the combine term, so fp8 there costs only ~1.5e-3 of
rel-l2 (3.3e-3 -> 4.8e-3, gate is 2e-2).  Stages 1/2 must stay bf16: fp8 on
either combine operand or the expert matmul measures 2.4e-2..5.6e-2.

Perf notes (measured on these cores): the PE is power-throttled
(throttle_gpio_2 active ~91%, util limit ~78%) — sustained bf16 cadence is
262 ns per 512-wide matmul vs the 213 ns unthrottled streaming time, with
LDWEIGHTS already hidden by hardware double-buffering.  The kernel is
therefore PE-cycle-bound, not schedule-bound; bf16 561 us -> with fp8
stage 3 measured ~329-340 us (run-to-run spread ±6 us from the throttle
and the cross-core start barrier).  Stage-1's t=40..130 us window runs at
~280-335 GB/s of the 358 GB/s DMA cap (ft/we feed + AllGather-0 + stage-3
full-residency prefetch) — moving the prefetch anywhere else measured
worse; see the inline comments.
"""

import os
import numpy as np
import ml_dtypes

E, C, D_IN, D_OUT = 8, 1024, 4096, 1024
B, S = 2, 2048
TOK = B * S                 # 4096 tokens
N_CORES = 8
S_LOC = TOK // N_CORES      # 512 tokens per core
CH = C // 2                 # c-half = 512
BF16 = ml_dtypes.bfloat16

MODE = os.environ.get("TRN_KERNEL_MODE", "bf16")
LDW_OPT = os.environ.get("TRN_LDW_OPT", "0") == "1"
SKIP_LDW = os.environ.get("TRN_SKIP_LDW", "1") == "1"
NO_MOVE_WAITS = os.environ.get("TRN_NO_MOVE_WAITS", "0") == "1"
# stage-3 residual matmul in fp8 DoubleRow (2 k-tiles/instr, 2x PE rate).
# The residual contribution is ~50x smaller than the combine term, so fp8
# quantization there is invisible (simulated rel-l2 3.35e-3 vs 3.25e-3).
S3FP8 = os.environ.get("TRN_S3FP8", "1") == "1"
F8 = ml_dtypes.float8_e4m3fn

_prog_cache = {}


def _patch_ldw_opt():
    """Compile this kernel's NEFF with walrus' LDWEIGHTS double-buffering
    (--enable-ldw-opt=true): hides the per-matmul 128-column weight load
    behind the previous matmul (~50 ns/MM here). Wrapped so only our
    compile is affected."""
    from concourse import bass_utils
    if getattr(bass_utils, "_ldw_opt_patched", False):
        return
    orig = bass_utils.run_command

    def patched(argv, **kw):
        argv = ["--enable-ldw-opt=true" if a == "--enable-ldw-opt=false" else a
                for a in argv]
        return orig(argv, **kw)

    bass_utils.run_command = patched
    bass_utils._ldw_opt_patched = True


def _build(mode, ldw_opt):
    import concourse.bass as bass  # noqa: F401
    import concourse.mybir as mybir
    from concourse import bacc
    from concourse.tile import TileContext, add_dep_helper

    dt = mybir.dt
    # fp32r must be the declared dtype end-to-end (the BIR verifier rejects
    # fp32-typed producers feeding fp32r matmuls), not a bitcast at the matmul
    io_dt = {"bf16": dt.bfloat16, "fp32r": dt.float32r, "fp32": dt.float32}[mode]
    mm_cast = lambda ap: ap

    nc = bacc.Bacc("TRN2", target_bir_lowering=False, debug=False, num_devices=N_CORES)
    if NO_MOVE_WAITS:
        # keep semaphore waits on the matmuls (not the ldweights) so walrus'
        # --enable-ldw-opt pass accepts the program; instance-level no-op
        nc.move_matmul_waits_to_ldweights = lambda: None

    s3_dt = dt.float8e4 if S3FP8 else io_dt
    fT = nc.declare_dram_parameter("fT", [D_IN, C], io_dt, isOutput=False)
    we = nc.declare_dram_parameter("we", [D_IN, D_OUT], io_dt, isOutput=False)
    cwT = nc.declare_dram_parameter("cwT", [E * C, S_LOC], io_dt, isOutput=False)
    riT = nc.declare_dram_parameter("riT", [D_IN, S_LOC], s3_dt, isOutput=False)
    rw = nc.declare_dram_parameter("rw", [D_IN, D_OUT], s3_dt, isOutput=False)
    out = nc.declare_dram_parameter("out", [S_LOC, D_OUT], dt.float32, isOutput=True)

    # variant tag in a tensor name so differently-compiled builds never share
    # a jax compile-cache entry
    nc.dram_tensor(f"variant_{mode}_{int(ldw_opt)}_{int(SKIP_LDW)}_{int(NO_MOVE_WAITS)}"
                   f"_{int(S3FP8)}", [1, 1], dt.float32)

    ag_in = [nc.dram_tensor(f"ag_in{h}", [CH, D_OUT], io_dt) for h in range(2)]
    ag_out = [nc.dram_tensor(f"ag_out{h}", [N_CORES * CH, D_OUT], io_dt,
                             addr_space="Shared") for h in range(2)]

    KT = D_IN // 128            # 32 contraction tiles
    SUB = 4                     # k-subtiles per DMA'd block
    NBLK = KT // SUB            # 8 blocks
    ECT = (E * C) // 128        # 64 combine contraction tiles
    NFREE = 512                 # ISA cap: s3d3_mm_num_elements <= 512
    NJ = D_OUT // NFREE
    rearr = lambda a: a.rearrange("(n p) d -> p n d", p=128)

    S1SUB = 2                   # finer stage-1 blocks: deeper prefetch pipeline
    S1BLK = KT // S1SUB         # 16 blocks

    bf16_mode = io_dt == dt.bfloat16
    B_FT, B_WE, B_RI, B_RW, B_CW, B_EOAG, B_EO = (
        (12, 9, 3, 3, 3, 6, 2) if bf16_mode else (4, 4, 2, 2, 2, 2, 1))
    if S3FP8 and bf16_mode:
        # fp8 ri/rw tiles are half-size; make them fully resident (8 blocks
        # each) so stage 3 runs with zero DMA waits — its loads prefetch
        # during stage 1, before AllGather chunk 1 saturates the DMA rings
        # (measured 9 us stage-3 stall with 3 bufs).
        B_FT, B_RI, B_RW = 12, 8, 8
    with TileContext(nc) as tc:
        with tc.tile_pool(name="p_ft", bufs=B_FT) as p_ft, \
             tc.tile_pool(name="p_we", bufs=B_WE) as p_we, \
             tc.tile_pool(name="p_ri", bufs=B_RI) as p_ri, \
             tc.tile_pool(name="p_rw", bufs=B_RW) as p_rw, \
             tc.tile_pool(name="p_cw", bufs=B_CW) as p_cw, \
             tc.tile_pool(name="p_eoag", bufs=B_EOAG) as p_eoag, \
             tc.tile_pool(name="p_eo", bufs=B_EO) as p_eo, \
             tc.tile_pool(name="p_out", bufs=1) as p_out, \
             tc.tile_pool(name="psum", bufs=1, space="PSUM") as p_ps:


            def mm_pair(psrow, lhsT_ap, rhs_of_j, start, stop):
                """Two matmuls sharing one stationary operand: the second
                skips its LDWEIGHTS (identical weights already in the array)
                and is order-pinned right after the first."""
                prev = None
                for j in range(NJ):
                    m = nc.tensor.matmul(psrow[j], lhsT_ap, rhs_of_j(j),
                                         start=start, stop=stop)
                    # fp32's two-pass matmul requires self-loading weights
                    if j > 0 and SKIP_LDW and bf16_mode:
                        m.ins.ldweights = False
                        add_dep_helper(m.ins, prev.ins, False, "weight-reuse pair order")
                    prev = m
                return prev

            def psum_tiles(tagp):
                return [[p_ps.tile([128, NFREE], dt.float32,
                                   name=f"{tagp}_{i}_{j}", tag=f"ps_{i}_{j}")
                         for j in range(NJ)] for i in range(4)]

            # Stage-3 fp8 operands are fully resident (8 bufs each): their
            # dma_starts sit at the stage-3 program position on sync/scalar,
            # which self-paces them in queue-FIFO order behind all stage-1
            # ft/we loads.  (Every attempt to move or pace them differently —
            # SWDGE burst, SWDGE paced, FIFO-interleaved with stage-1 loads —
            # measured 6-10 us WORSE by starving stage-1's just-in-time feed.)
            rearr2 = lambda a: a.rearrange("(s t p) d -> p s t d", p=128, t=2)

            # ------------- Stage 1: eo_e = fT.T @ we, by c-halves ------------
            last_we_dma = None
            for ch in range(2):
                c0 = ch * CH
                psums = psum_tiles(f"s1h{ch}")
                for blk in range(S1BLK):
                    r0 = blk * 128 * S1SUB
                    # only this c-half's columns of fT are needed
                    ft_t = p_ft.tile([128, S1SUB, CH], io_dt, tag="ft", name=f"ft_{ch}_{blk}")
                    we_t = p_we.tile([128, S1SUB, D_OUT], io_dt, tag="we", name=f"we_{ch}_{blk}")
                    if ch == 0 and blk == 0:
                        # per-sub loads: the first matmul starts after 256 KiB,
                        # not after the whole block.  (Splitting these finer
                        # — per-256-col pieces — measured WORSE: the extra
                        # trigger instructions at the queue head delay every
                        # subsequent transfer.)
                        for sub in range(S1SUB):
                            nc.sync.dma_start(
                                out=ft_t[:, sub:sub + 1, :],
                                in_=rearr(fT[r0 + sub * 128:r0 + (sub + 1) * 128, c0:c0 + CH]))
                            last_we_dma = nc.scalar.dma_start(
                                out=we_t[:, sub:sub + 1, :],
                                in_=rearr(we[r0 + sub * 128:r0 + (sub + 1) * 128, :]))
                    else:
                        nc.sync.dma_start(
                            out=ft_t, in_=rearr(fT[r0:r0 + 128 * S1SUB, c0:c0 + CH]))
                        # scalar queue: second HWDGE ring, parallel with sync's
                        last_we_dma = nc.scalar.dma_start(
                            out=we_t, in_=rearr(we[r0:r0 + 128 * S1SUB, :]))
                    for sub in range(S1SUB):
                        kt = blk * S1SUB + sub
                        for i in range(4):
                            mm_pair(psums[i],
                                    mm_cast(ft_t[:, sub, i * 128:(i + 1) * 128]),
                                    lambda j, sub=sub: mm_cast(we_t[:, sub, j * NFREE:(j + 1) * NFREE]),
                                    start=(kt == 0), stop=(kt == KT - 1))

                eo_half = p_eo.tile([128, 4, D_OUT], io_dt, tag="eo", name=f"eo_{ch}")
                for i in range(4):
                    for j in range(NJ):
                        nc.vector.tensor_copy(out=eo_half[:, i, j * NFREE:(j + 1) * NFREE],
                                              in_=psums[i][j])
                # gpsimd (SWDGE) queue: keeps this late-gated write out of the
                # HWDGE FIFOs so it can't head-of-line block operand loads
                nc.gpsimd.dma_start(out=rearr(ag_in[ch][:]), in_=eo_half)
                # chunked AllGather: starts while the PE grinds the next phase
                nc.gpsimd.collective_compute(
                    "AllGather", mybir.AluOpType.bypass,
                    replica_groups=[list(range(N_CORES))],
                    ins=[ag_in[ch][:].opt()], outs=[ag_out[ch][:].opt()])

            # ------------- Stage 3: resid partial (w1 folded) ----------------
            psums = psum_tiles("s23")
            last_rw_dma = None
            if S3FP8:
                # fp8 DoubleRow: each matmul contracts TWO 128-k tiles
                # (lhsT [128,2,128], rhs [128,2,256] -> psum [128,256]),
                # halving the PE-active cycles of this stage.
                for blk in range(NBLK):
                    ri_t = p_ri.tile([128, 2, 2, S_LOC], s3_dt, tag="ri",
                                     name=f"ri_{blk}")
                    nc.sync.dma_start(
                        out=ri_t, in_=rearr2(riT[blk * 512:(blk + 1) * 512, :]))
                    rw_t = p_rw.tile([128, 2, 2, D_OUT], s3_dt, tag="rw",
                                     name=f"rw_{blk}")
                    nc.scalar.dma_start(
                        out=rw_t, in_=rearr2(rw[blk * 512:(blk + 1) * 512, :]))
                    for sub in range(2):
                        first = blk == 0 and sub == 0
                        for i in range(4):
                            for n in range(4):
                                nc.tensor.matmul(
                                    psums[i][n // 2][:, (n % 2) * 256:(n % 2) * 256 + 256],
                                    ri_t[:, sub, :, i * 128:(i + 1) * 128],
                                    rw_t[:, sub, :, n * 256:(n + 1) * 256],
                                    start=first, stop=False,
                                    perf_mode=mybir.MatmulPerfMode.DoubleRow,
                                    skip_group_check=True)
            else:
                for blk in range(NBLK):
                    ri_t = p_ri.tile([128, SUB, S_LOC], io_dt, tag="ri", name=f"ri_{blk}")
                    nc.sync.dma_start(out=ri_t, in_=rearr(riT[blk * 512:(blk + 1) * 512, :]))
                    rw_t = p_rw.tile([128, SUB, D_OUT], io_dt, tag="rw", name=f"rw_{blk}")
                    last_rw_dma = nc.scalar.dma_start(
                        out=rw_t, in_=rearr(rw[blk * 512:(blk + 1) * 512, :]))
                    for sub in range(SUB):
                        kt = blk * SUB + sub
                        for i in range(4):
                            mm_pair(psums[i],
                                    mm_cast(ri_t[:, sub, i * 128:(i + 1) * 128]),
                                    lambda j, sub=sub: mm_cast(rw_t[:, sub, j * NFREE:(j + 1) * NFREE]),
                                    start=(kt == 0), stop=False)

            # ------------- Stage 2: combine partial (w0 folded) --------------
            out_sb = p_out.tile([128, 4, D_OUT], dt.float32)
            prev_eoag_dma = None
            for blk in range(ECT // SUB):
                half = blk // 8              # ag chunk this block reads
                r0 = (blk % 8) * 512
                cw_t = p_cw.tile([128, SUB, S_LOC], io_dt, tag="cw", name=f"cw_{blk}")
                nc.sync.dma_start(out=cw_t, in_=rearr(cwT[blk * 512:(blk + 1) * 512, :]))
                eo_t = p_eoag.tile([128, SUB, D_OUT], io_dt, tag="eoag", name=f"eoag_{blk}")
                eoag_dma = nc.scalar.dma_start(out=eo_t, in_=rearr(ag_out[half][r0:r0 + 512, :]))
                # Scalar-queue order: eoag loads wait on the AllGathers, so pin
                # them after every stage-1/3 operand load and in block order —
                # otherwise the scheduler can hoist one ahead and head-of-line
                # block the HWDGE FIFO on the collective (measured 44 us stall).
                prev = prev_eoag_dma if prev_eoag_dma is not None else (
                    last_rw_dma if last_rw_dma is not None else last_we_dma)
                if prev is not None:
                    add_dep_helper(eoag_dma.ins, prev.ins, False,
                                   "eoag after stage-1/3 loads, in block order")
                prev_eoag_dma = eoag_dma
                last_blk = blk == ECT // SUB - 1
                if not last_blk:
                    for sub in range(SUB):
                        for i in range(4):
                            mm_pair(psums[i],
                                    mm_cast(cw_t[:, sub, i * 128:(i + 1) * 128]),
                                    lambda j, sub=sub: mm_cast(eo_t[:, sub, j * NFREE:(j + 1) * NFREE]),
                                    start=False, stop=False)
                else:
                    # last block: finish groups one at a time so the PSUM->SBUF
                    # copies and output DMAs overlap the remaining matmuls
                    for i in range(4):
                        for j in range(NJ):
                            for sub in range(SUB):
                                nc.tensor.matmul(
                                    psums[i][j],
                                    mm_cast(cw_t[:, sub, i * 128:(i + 1) * 128]),
                                    mm_cast(eo_t[:, sub, j * NFREE:(j + 1) * NFREE]),
                                    start=False, stop=(sub == SUB - 1))
                            nc.vector.tensor_copy(
                                out=out_sb[:, i, j * NFREE:(j + 1) * NFREE],
                                in_=psums[i][j])
                        nc.sync.dma_start(
                            out=out[i * 128:(i + 1) * 128, :].rearrange("(n p) d -> p n d", p=128),
                            in_=out_sb[:, i:i + 1, :])

    nc.finalize()
    return nc


def _get_prog(mode, ldw_opt):
    key = (mode, ldw_opt)
    if key not in _prog_cache:
        if ldw_opt:
            _patch_ldw_opt()
        _prog_cache[key] = _build(mode, ldw_opt)
    return _prog_cache[key]


def _prep_in_maps(inputs, expert_w, residual_w, combine_weights, residual_weight, mode):
    np_dt = BF16 if mode == "bf16" else np.float32
    s3_dt = F8 if S3FP8 else np_dt
    front = inputs[:E * C].reshape(E, C, D_IN)
    resid = inputs[E * C:]                       # [TOK, D_IN]
    rwt = residual_weight.reshape(TOK, 2)
    w0, w1 = rwt[:, 0], rwt[:, 1]

    rw_cast = np.ascontiguousarray(residual_w.astype(s3_dt))
    resid_s = resid * w1[:, None]                # fold w1 (fp32)
    in_maps = []
    for r in range(N_CORES):
        sl = slice(r * S_LOC, (r + 1) * S_LOC)
        fT = np.ascontiguousarray(front[r].T.astype(np_dt))              # [D_IN, C]
        we = np.ascontiguousarray(expert_w[r].astype(np_dt))             # [D_IN, D_OUT]
        cw_s = combine_weights[sl] * w0[sl, None, None]                  # [S_LOC, E, C]
        # contraction rows ordered (c-half chunk, expert, c-within-half) to
        # match the chunked AllGather's concatenation
        cwT = np.ascontiguousarray(
            cw_s.reshape(S_LOC, E, 2, CH).transpose(2, 1, 3, 0).reshape(E * C, S_LOC)
            .astype(np_dt))
        riT = np.ascontiguousarray(resid_s[sl].T.astype(s3_dt))          # [D_IN, S_LOC]
        in_maps.append({"fT": fT, "we": we, "cwT": cwT, "riT": riT, "rw": rw_cast})
    return in_maps


def _run(inputs, expert_w, expert_b, residual_w, residual_b,
         combine_weights, residual_weight, mode=None, ldw_opt=None, trace=False):
    import jax
    try:
        if jax.config.jax_compilation_cache_dir is None:
            jax.config.update("jax_compilation_cache_dir", "/tmp/jax_cache_trn_moe")
            jax.config.update("jax_persistent_cache_min_compile_time_secs", 0.5)
    except Exception:
        pass
    from concourse.bass_utils import run_bass_kernel_spmd

    mode = mode or MODE
    ldw_opt = LDW_OPT if ldw_opt is None else ldw_opt
    inputs = np.asarray(inputs, dtype=np.float32)
    expert_w = np.asarray(expert_w, dtype=np.float32)
    expert_b = np.asarray(expert_b, dtype=np.float32)
    residual_w = np.asarray(residual_w, dtype=np.float32)
    residual_b = np.asarray(residual_b, dtype=np.float32)
    combine_weights = np.asarray(combine_weights, dtype=np.float32)
    residual_weight = np.asarray(residual_weight, dtype=np.float32)

    nc = _get_prog(mode, ldw_opt)
    in_maps = _prep_in_maps(inputs, expert_w, residual_w, combine_weights,
                            residual_weight, mode)
    res = run_bass_kernel_spmd(nc, in_maps, list(range(N_CORES)), trace=trace)
    out = np.concatenate([res.results[r]["out"] for r in range(N_CORES)], axis=0)

    # exact bias contributions (zero in practice, but keep the math honest)
    rwt = residual_weight.reshape(TOK, 2)
    if residual_b.any():
        out = out + rwt[:, 1:2] * residual_b[None, :]
    if expert_b.any():
        cs = combine_weights.sum(axis=2)                    # [TOK, E]
        out = out + rwt[:, 0:1] * (cs @ expert_b)
    return out.reshape(B, S, D_OUT).astype(np.float32), res


def kernel(**kw):
    out, _ = _run(**kw)
    return out



# revision 32
# speedup vs baseline: 1.0699x; 1.0675x over previous
"""Expert-parallel MoE "behind" block + residual on 8 Trainium2 NeuronCores.

Reference computation (fp32):
    front      = inputs[:E*C].reshape(E, C, D_IN)
    expert_out = einsum("ecd,edm->ecm", front, expert_w) + expert_b
    combined   = einsum("sec,ecm->sm", combine_weights, expert_out)
    resid      = inputs[E*C:] @ residual_w + residual_b
    out        = combined * w0[:, None] + resid * w1[:, None]

Sharding (8 cores):
  Stage 1 (expert-parallel): core e computes eo_e = front_e @ W_e  [C, D_OUT],
  in two c-halves; each half is AllGathered over the cores as soon as it is
  ready (2 chunked AllGathers overlap stage-1/3 compute on the PE).
  Stage 3 (token-parallel residual): core r owns tokens S_r (512 rows) and
  accumulates (w1*resid)[S_r] @ residual_w into 8 PSUM banks.
  Stage 2 (token-parallel combine): accumulates (w0*cw)[S_r] @ eo_full into
  the same PSUM banks (w0/w1 folded into cw / resid rows on the host; exact).
  The (all-zero) bias terms are added back exactly on the host:
      out += w1 x residual_b  +  w0 * (cw.sum(c) @ expert_b)

All device matmuls contract over the SBUF partition axis, so every DRAM
operand is laid out contraction-major on the host.  The chunked AllGather
concatenates per-rank c-halves, so cwT's contraction rows are ordered
(chunk, expert, c-within-half) to match.

Modes (env TRN_KERNEL_MODE): "bf16" (default) ships bf16 operands with fp32
PSUM accumulate.  "fp32" is the exact fallback (plain fp32 PE at 4
cycles/row) — 1.37 ms, rel-l2 6.6e-7.  "fp32r" compiles but mis-computes on
this hardware (15% error) — do not use.

Stage 3 runs in fp8-e4m3 DoubleRow perf mode (TRN_S3FP8=1, default): each
matmul contracts two 128-k tiles at 0.5 PE cycles per output element,
halving that stage's PE-active time (67 -> 34 us).  The residual term is
~50x smaller than the combine term, so fp8 there costs only ~1.5e-3 of
rel-l2 (3.3e-3 -> 4.8e-3, gate is 2e-2).  Stages 1/2 must stay bf16: fp8 on
either combine operand or the expert matmul measures 2.4e-2..5.6e-2.

Perf notes (measured on these cores): the PE is power-throttled
(throttle_gpio_2 active ~91%, util limit ~78%) — sustained bf16 cadence is
262 ns per 512-wide matmul vs the 213 ns unthrottled streaming time, with
LDWEIGHTS already hidden by hardware double-buffering.  The kernel is
therefore PE-cycle-bound, not schedule-bound; bf16 561 us -> with fp8
stage 3 measured ~329-340 us (run-to-run spread ±6 us from the throttle
and the cross-core start barrier).  Stage-1's t=40..130 us window runs at
~280-335 GB/s of the 358 GB/s DMA cap (ft/we feed + AllGather-0 + stage-3
full-residency prefetch) — moving the prefetch anywhere else measured
worse; see the inline comments.
"""

import os
import numpy as np
import ml_dtypes

E, C, D_IN, D_OUT = 8, 1024, 4096, 1024
B, S = 2, 2048
TOK = B * S                 # 4096 tokens
N_CORES = 8
S_LOC = TOK // N_CORES      # 512 tokens per core
CH = C // 2                 # c-half = 512
BF16 = ml_dtypes.bfloat16

MODE = os.environ.get("TRN_KERNEL_MODE", "bf16")
LDW_OPT = os.environ.get("TRN_LDW_OPT", "0") == "1"
SKIP_LDW = os.environ.get("TRN_SKIP_LDW", "1") == "1"
NO_MOVE_WAITS = os.environ.get("TRN_NO_MOVE_WAITS", "0") == "1"
# stage-3 residual matmul in fp8 DoubleRow (2 k-tiles/instr, 2x PE rate).
# The residual contribution is ~50x smaller than the combine term, so fp8
# quantization there is invisible (simulated rel-l2 3.35e-3 vs 3.25e-3).
S3FP8 = os.environ.get("TRN_S3FP8", "1") == "1"
F8 = ml_dtypes.float8_e4m3fn

_prog_cache = {}


def _patch_ldw_opt():
    """Compile this kernel's NEFF with walrus' LDWEIGHTS double-buffering
    (--enable-ldw-opt=true): hides the per-matmul 128-column weight load
    behind the previous matmul (~50 ns/MM here). Wrapped so only our
    compile is affected."""
    from concourse import bass_utils
    if getattr(bass_utils, "_ldw_opt_patched", False):
        return
    orig = bass_utils.run_command

    def patched(argv, **kw):
        argv = ["--enable-ldw-opt=true" if a == "--enable-ldw-opt=false" else a
                for a in argv]
        return orig(argv, **kw)

    bass_utils.run_command = patched
    bass_utils._ldw_opt_patched = True


def _build(mode, ldw_opt):
    import concourse.bass as bass  # noqa: F401
    import concourse.mybir as mybir
    from concourse import bacc
    from concourse.tile import TileContext, add_dep_helper

    dt = mybir.dt
    # fp32r must be the declared dtype end-to-end (the BIR verifier rejects
    # fp32-typed producers feeding fp32r matmuls), not a bitcast at the matmul
    io_dt = {"bf16": dt.bfloat16, "fp32r": dt.float32r, "fp32": dt.float32}[mode]
    mm_cast = lambda ap: ap

    nc = bacc.Bacc("TRN2", target_bir_lowering=False, debug=False, num_devices=N_CORES)
    if NO_MOVE_WAITS:
        # keep semaphore waits on the matmuls (not the ldweights) so walrus'
        # --enable-ldw-opt pass accepts the program; instance-level no-op
        nc.move_matmul_waits_to_ldweights = lambda: None

    s3_dt = dt.float8e4 if S3FP8 else io_dt
    fT = nc.declare_dram_parameter("fT", [D_IN, C], io_dt, isOutput=False)
    we = nc.declare_dram_parameter("we", [D_IN, D_OUT], io_dt, isOutput=False)
    cwT = nc.declare_dram_parameter("cwT", [E * C, S_LOC], io_dt, isOutput=False)
    riT = nc.declare_dram_parameter("riT", [D_IN, S_LOC], s3_dt, isOutput=False)
    rw = nc.declare_dram_parameter("rw", [D_IN, D_OUT], s3_dt, isOutput=False)
    out = nc.declare_dram_parameter("out", [S_LOC, D_OUT], dt.float32, isOutput=True)

    # variant tag in a tensor name so differently-compiled builds never share
    # a jax compile-cache entry
    nc.dram_tensor(f"variant_{mode}_{int(ldw_opt)}_{int(SKIP_LDW)}_{int(NO_MOVE_WAITS)}"
                   f"_{int(S3FP8)}", [1, 1], dt.float32)

    ag_in = [nc.dram_tensor(f"ag_in{h}", [CH, D_OUT], io_dt) for h in range(2)]
    ag_out = [nc.dram_tensor(f"ag_out{h}", [N_CORES * CH, D_OUT], io_dt,
                             addr_space="Shared") for h in range(2)]

    KT = D_IN // 128            # 32 contraction tiles
    SUB = 4                     # k-subtiles per DMA'd block
    NBLK = KT // SUB            # 8 blocks
    ECT = (E * C) // 128        # 64 combine contraction tiles
    NFREE = 512                 # ISA cap: s3d3_mm_num_elements <= 512
    NJ = D_OUT // NFREE
    rearr = lambda a: a.rearrange("(n p) d -> p n d", p=128)

    S1SUB = 2                   # finer stage-1 blocks: deeper prefetch pipeline
    S1BLK = KT // S1SUB         # 16 blocks

    bf16_mode = io_dt == dt.bfloat16
    B_FT, B_WE, B_RI, B_RW, B_CW, B_EOAG, B_EO = (
        (12, 9, 3, 3, 3, 6, 2) if bf16_mode else (4, 4, 2, 2, 2, 2, 1))
    if S3FP8 and bf16_mode:
        # fp8 ri/rw tiles are half-size; make them fully resident (8 blocks
        # each) so stage 3 runs with zero DMA waits — its loads prefetch
        # during stage 1, before AllGather chunk 1 saturates the DMA rings
        # (measured 9 us stage-3 stall with 3 bufs).  B_WE=16 makes the
        # expert weights fully resident so c-half 1 skips the 8 MB re-read.
        B_FT, B_WE, B_RI, B_RW, B_EOAG = 9, 16, 8, 8, 4
    with TileContext(nc) as tc:
        with tc.tile_pool(name="p_ft", bufs=B_FT) as p_ft, \
             tc.tile_pool(name="p_we", bufs=B_WE) as p_we, \
             tc.tile_pool(name="p_ri", bufs=B_RI) as p_ri, \
             tc.tile_pool(name="p_rw", bufs=B_RW) as p_rw, \
             tc.tile_pool(name="p_cw", bufs=B_CW) as p_cw, \
             tc.tile_pool(name="p_eoag", bufs=B_EOAG) as p_eoag, \
             tc.tile_pool(name="p_eo", bufs=B_EO) as p_eo, \
             tc.tile_pool(name="p_out", bufs=1) as p_out, \
             tc.tile_pool(name="psum", bufs=1, space="PSUM") as p_ps:


            def mm_pair(psrow, lhsT_ap, rhs_of_j, start, stop):
                """Two matmuls sharing one stationary operand: the second
                skips its LDWEIGHTS (identical weights already in the array)
                and is order-pinned right after the first."""
                prev = None
                for j in range(NJ):
                    m = nc.tensor.matmul(psrow[j], lhsT_ap, rhs_of_j(j),
                                         start=start, stop=stop)
                    # fp32's two-pass matmul requires self-loading weights
                    if j > 0 and SKIP_LDW and bf16_mode:
                        m.ins.ldweights = False
                        add_dep_helper(m.ins, prev.ins, False, "weight-reuse pair order")
                    prev = m
                return prev

            def psum_tiles(tagp):
                return [[p_ps.tile([128, NFREE], dt.float32,
                                   name=f"{tagp}_{i}_{j}", tag=f"ps_{i}_{j}")
                         for j in range(NJ)] for i in range(4)]

            # Stage-3 fp8 operands are fully resident (8 bufs each): their
            # dma_starts sit at the stage-3 program position on sync/scalar,
            # which self-paces them in queue-FIFO order behind all stage-1
            # ft/we loads.  (Every attempt to move or pace them differently —
            # SWDGE burst, SWDGE paced, FIFO-interleaved with stage-1 loads —
            # measured 6-10 us WORSE by starving stage-1's just-in-time feed.)
            rearr2 = lambda a: a.rearrange("(s t p) d -> p s t d", p=128, t=2)

            # ------------- Stage 1: eo_e = fT.T @ we, by c-halves ------------
            # we (the expert weight matrix) is read by BOTH c-halves.  When
            # B_WE >= S1BLK it is loaded once and kept SBUF-resident, cutting
            # stage-1 DMA from 24 MB to 16 MB — the t=40..130 us window runs
            # at the DMA bandwidth cap, and half-1's 8 MB we re-read was a
            # direct contributor to stage-1's just-in-time feed stalls.
            we_resident = B_WE >= S1BLK
            we_tiles = {}
            last_we_dma = None
            for ch in range(2):
                c0 = ch * CH
                psums = psum_tiles(f"s1h{ch}")
                for blk in range(S1BLK):
                    r0 = blk * 128 * S1SUB
                    # only this c-half's columns of fT are needed
                    ft_t = p_ft.tile([128, S1SUB, CH], io_dt, tag="ft", name=f"ft_{ch}_{blk}")
                    load_we = not (we_resident and ch == 1)
                    if load_we:
                        we_t = p_we.tile([128, S1SUB, D_OUT], io_dt, tag="we",
                                         name=f"we_{ch}_{blk}")
                        we_tiles[blk] = we_t
                    else:
                        we_t = we_tiles[blk]
                    if ch == 0 and blk == 0:
                        # per-sub loads: the first matmul starts after 256 KiB,
                        # not after the whole block.  (Splitting these finer
                        # — per-256-col pieces — measured WORSE: the extra
                        # trigger instructions at the queue head delay every
                        # subsequent transfer.)
                        for sub in range(S1SUB):
                            nc.sync.dma_start(
                                out=ft_t[:, sub:sub + 1, :],
                                in_=rearr(fT[r0 + sub * 128:r0 + (sub + 1) * 128, c0:c0 + CH]))
                            last_we_dma = nc.scalar.dma_start(
                                out=we_t[:, sub:sub + 1, :],
                                in_=rearr(we[r0 + sub * 128:r0 + (sub + 1) * 128, :]))
                    else:
                        nc.sync.dma_start(
                            out=ft_t, in_=rearr(fT[r0:r0 + 128 * S1SUB, c0:c0 + CH]))
                        if load_we:
                            # scalar queue: second HWDGE ring, parallel with sync's
                            last_we_dma = nc.scalar.dma_start(
                                out=we_t, in_=rearr(we[r0:r0 + 128 * S1SUB, :]))
                    for sub in range(S1SUB):
                        kt = blk * S1SUB + sub
                        for i in range(4):
                            mm_pair(psums[i],
                                    mm_cast(ft_t[:, sub, i * 128:(i + 1) * 128]),
                                    lambda j, sub=sub: mm_cast(we_t[:, sub, j * NFREE:(j + 1) * NFREE]),
                                    start=(kt == 0), stop=(kt == KT - 1))

                eo_half = p_eo.tile([128, 4, D_OUT], io_dt, tag="eo", name=f"eo_{ch}")
                for i in range(4):
                    for j in range(NJ):
                        nc.vector.tensor_copy(out=eo_half[:, i, j * NFREE:(j + 1) * NFREE],
                                              in_=psums[i][j])
                # gpsimd (SWDGE) queue: keeps this late-gated write out of the
                # HWDGE FIFOs so it can't head-of-line block operand loads
                nc.gpsimd.dma_start(out=rearr(ag_in[ch][:]), in_=eo_half)
                # chunked AllGather: starts while the PE grinds the next phase
                nc.gpsimd.collective_compute(
                    "AllGather", mybir.AluOpType.bypass,
                    replica_groups=[list(range(N_CORES))],
                    ins=[ag_in[ch][:].opt()], outs=[ag_out[ch][:].opt()])

            # ------------- Stage 3: resid partial (w1 folded) ----------------
            psums = psum_tiles("s23")
            last_rw_dma = None
            if S3FP8:
                # fp8 DoubleRow: each matmul contracts TWO 128-k tiles
                # (lhsT [128,2,128], rhs [128,2,256] -> psum [128,256]),
                # halving the PE-active cycles of this stage.
                for blk in range(NBLK):
                    ri_t = p_ri.tile([128, 2, 2, S_LOC], s3_dt, tag="ri",
                                     name=f"ri_{blk}")
                    nc.sync.dma_start(
                        out=ri_t, in_=rearr2(riT[blk * 512:(blk + 1) * 512, :]))
                    rw_t = p_rw.tile([128, 2, 2, D_OUT], s3_dt, tag="rw",
                                     name=f"rw_{blk}")
                    # track for the eoag ordering chain: eoag loads must not
                    # hoist ahead of these on the scalar ring
                    last_rw_dma = nc.scalar.dma_start(
                        out=rw_t, in_=rearr2(rw[blk * 512:(blk + 1) * 512, :]))
                    for sub in range(2):
                        first = blk == 0 and sub == 0
                        for i in range(4):
                            for n in range(4):
                                nc.tensor.matmul(
                                    psums[i][n // 2][:, (n % 2) * 256:(n % 2) * 256 + 256],
                                    ri_t[:, sub, :, i * 128:(i + 1) * 128],
                                    rw_t[:, sub, :, n * 256:(n + 1) * 256],
                                    start=first, stop=False,
                                    perf_mode=mybir.MatmulPerfMode.DoubleRow,
                                    skip_group_check=True)
            else:
                for blk in range(NBLK):
                    ri_t = p_ri.tile([128, SUB, S_LOC], io_dt, tag="ri", name=f"ri_{blk}")
                    nc.sync.dma_start(out=ri_t, in_=rearr(riT[blk * 512:(blk + 1) * 512, :]))
                    rw_t = p_rw.tile([128, SUB, D_OUT], io_dt, tag="rw", name=f"rw_{blk}")
                    last_rw_dma = nc.scalar.dma_start(
                        out=rw_t, in_=rearr(rw[blk * 512:(blk + 1) * 512, :]))
                    for sub in range(SUB):
                        kt = blk * SUB + sub
                        for i in range(4):
                            mm_pair(psums[i],
                                    mm_cast(ri_t[:, sub, i * 128:(i + 1) * 128]),
                                    lambda j, sub=sub: mm_cast(rw_t[:, sub, j * NFREE:(j + 1) * NFREE]),
                                    start=(kt == 0), stop=False)

            # ------------- Stage 2: combine partial (w0 folded) --------------
            out_sb = p_out.tile([128, 4, D_OUT], dt.float32)
            prev_eoag_dma = None
            for blk in range(ECT // SUB):
                half = blk // 8              # ag chunk this block reads
                r0 = (blk % 8) * 512
                cw_t = p_cw.tile([128, SUB, S_LOC], io_dt, tag="cw", name=f"cw_{blk}")
                nc.sync.dma_start(out=cw_t, in_=rearr(cwT[blk * 512:(blk + 1) * 512, :]))
                eo_t = p_eoag.tile([128, SUB, D_OUT], io_dt, tag="eoag", name=f"eoag_{blk}")
                eoag_dma = nc.scalar.dma_start(out=eo_t, in_=rearr(ag_out[half][r0:r0 + 512, :]))
                # Scalar-queue order: eoag loads wait on the AllGathers, so pin
                # them after every stage-1/3 operand load and in block order —
                # otherwise the scheduler can hoist one ahead and head-of-line
                # block the HWDGE FIFO on the collective (measured 44 us stall).
                prev = prev_eoag_dma if prev_eoag_dma is not None else (
                    last_rw_dma if last_rw_dma is not None else last_we_dma)
                if prev is not None:
                    add_dep_helper(eoag_dma.ins, prev.ins, False,
                                   "eoag after stage-1/3 loads, in block order")
                prev_eoag_dma = eoag_dma
                last_blk = blk == ECT // SUB - 1
                if not last_blk:
                    for sub in range(SUB):
                        for i in range(4):
                            mm_pair(psums[i],
                                    mm_cast(cw_t[:, sub, i * 128:(i + 1) * 128]),
                                    lambda j, sub=sub: mm_cast(eo_t[:, sub, j * NFREE:(j + 1) * NFREE]),
                                    start=False, stop=False)
                else:
                    # last block: finish groups one at a time so the PSUM->SBUF
                    # copies and output DMAs overlap the remaining matmuls
                    for i in range(4):
                        for j in range(NJ):
                            for sub in range(SUB):
                                nc.tensor.matmul(
                                    psums[i][j],
                                    mm_cast(cw_t[:, sub, i * 128:(i + 1) * 128]),
                                    mm_cast(eo_t[:, sub, j * NFREE:(j + 1) * NFREE]),
                                    start=False, stop=(sub == SUB - 1))
                            nc.vector.tensor_copy(
                                out=out_sb[:, i, j * NFREE:(j + 1) * NFREE],
                                in_=psums[i][j])
                        nc.sync.dma_start(
                            out=out[i * 128:(i + 1) * 128, :].rearrange("(n p) d -> p n d", p=128),
                            in_=out_sb[:, i:i + 1, :])

    nc.finalize()
    return nc


def _get_prog(mode, ldw_opt):
    key = (mode, ldw_opt)
    if key not in _prog_cache:
        if ldw_opt:
            _patch_ldw_opt()
        _prog_cache[key] = _build(mode, ldw_opt)
    return _prog_cache[key]


def _prep_in_maps(inputs, expert_w, residual_w, combine_weights, residual_weight, mode):
    np_dt = BF16 if mode == "bf16" else np.float32
    s3_dt = F8 if S3FP8 else np_dt
    front = inputs[:E * C].reshape(E, C, D_IN)
    resid = inputs[E * C:]                       # [TOK, D_IN]
    rwt = residual_weight.reshape(TOK, 2)
    w0, w1 = rwt[:, 0], rwt[:, 1]

    rw_cast = np.ascontiguousarray(residual_w.astype(s3_dt))
    resid_s = resid * w1[:, None]                # fold w1 (fp32)
    in_maps = []
    for r in range(N_CORES):
        sl = slice(r * S_LOC, (r + 1) * S_LOC)
        fT = np.ascontiguousarray(front[r].T.astype(np_dt))              # [D_IN, C]
        we = np.ascontiguousarray(expert_w[r].astype(np_dt))             # [D_IN, D_OUT]
        cw_s = combine_weights[sl] * w0[sl, None, None]                  # [S_LOC, E, C]
        # contraction rows ordered (c-half chunk, expert, c-within-half) to
        # match the chunked AllGather's concatenation
        cwT = np.ascontiguousarray(
            cw_s.reshape(S_LOC, E, 2, CH).transpose(2, 1, 3, 0).reshape(E * C, S_LOC)
            .astype(np_dt))
        riT = np.ascontiguousarray(resid_s[sl].T.astype(s3_dt))          # [D_IN, S_LOC]
        in_maps.append({"fT": fT, "we": we, "cwT": cwT, "riT": riT, "rw": rw_cast})
    return in_maps


def _run(inputs, expert_w, expert_b, residual_w, residual_b,
         combine_weights, residual_weight, mode=None, ldw_opt=None, trace=False):
    import jax
    try:
        if jax.config.jax_compilation_cache_dir is None:
            jax.config.update("jax_compilation_cache_dir", "/tmp/jax_cache_trn_moe")
            jax.config.update("jax_persistent_cache_min_compile_time_secs", 0.5)
    except Exception:
        pass
    from concourse.bass_utils import run_bass_kernel_spmd

    mode = mode or MODE
    ldw_opt = LDW_OPT if ldw_opt is None else ldw_opt
    inputs = np.asarray(inputs, dtype=np.float32)
    expert_w = np.asarray(expert_w, dtype=np.float32)
    expert_b = np.asarray(expert_b, dtype=np.float32)
    residual_w = np.asarray(residual_w, dtype=np.float32)
    residual_b = np.asarray(residual_b, dtype=np.float32)
    combine_weights = np.asarray(combine_weights, dtype=np.float32)
    residual_weight = np.asarray(residual_weight, dtype=np.float32)

    nc = _get_prog(mode, ldw_opt)
    in_maps = _prep_in_maps(inputs, expert_w, residual_w, combine_weights,
                            residual_weight, mode)
    res = run_bass_kernel_spmd(nc, in_maps, list(range(N_CORES)), trace=trace)
    out = np.concatenate([res.results[r]["out"] for r in range(N_CORES)], axis=0)

    # exact bias contributions (zero in practice, but keep the math honest)
    rwt = residual_weight.reshape(TOK, 2)
    if residual_b.any():
        out = out + rwt[:, 1:2] * residual_b[None, :]
    if expert_b.any():
        cs = combine_weights.sum(axis=2)                    # [TOK, E]
        out = out + rwt[:, 0:1] * (cs @ expert_b)
    return out.reshape(B, S, D_OUT).astype(np.float32), res


def kernel(**kw):
    out, _ = _run(**kw)
    return out



# revision 33
# speedup vs baseline: 1.0721x; 1.0021x over previous
"""Expert-parallel MoE "behind" block + residual on 8 Trainium2 NeuronCores.

Reference computation (fp32):
    front      = inputs[:E*C].reshape(E, C, D_IN)
    expert_out = einsum("ecd,edm->ecm", front, expert_w) + expert_b
    combined   = einsum("sec,ecm->sm", combine_weights, expert_out)
    resid      = inputs[E*C:] @ residual_w + residual_b
    out        = combined * w0[:, None] + resid * w1[:, None]

Sharding (8 cores):
  Stage 1 (expert-parallel): core e computes eo_e = front_e @ W_e  [C, D_OUT],
  in two c-halves; each half is AllGathered over the cores as soon as it is
  ready (2 chunked AllGathers overlap stage-1/3 compute on the PE).
  Stage 3 (token-parallel residual): core r owns tokens S_r (512 rows) and
  accumulates (w1*resid)[S_r] @ residual_w into 8 PSUM banks.
  Stage 2 (token-parallel combine): accumulates (w0*cw)[S_r] @ eo_full into
  the same PSUM banks (w0/w1 folded into cw / resid rows on the host; exact).
  The (all-zero) bias terms are added back exactly on the host:
      out += w1 x residual_b  +  w0 * (cw.sum(c) @ expert_b)

All device matmuls contract over the SBUF partition axis, so every DRAM
operand is laid out contraction-major on the host.  The chunked AllGather
concatenates per-rank c-halves, so cwT's contraction rows are ordered
(chunk, expert, c-within-half) to match.

Modes (env TRN_KERNEL_MODE): "bf16" (default) ships bf16 operands with fp32
PSUM accumulate.  "fp32" is the exact fallback (plain fp32 PE at 4
cycles/row) — 1.37 ms, rel-l2 6.6e-7.  "fp32r" compiles but mis-computes on
this hardware (15% error) — do not use.

Stage 3 runs in fp8-e4m3 DoubleRow perf mode (TRN_S3FP8=1, default): each
matmul contracts two 128-k tiles at 0.5 PE cycles per output element,
halving that stage's PE-active time (67 -> 34 us).  The residual term is
~50x smaller than the combine term, so fp8 there costs only ~1.5e-3 of
rel-l2 (3.3e-3 -> 4.8e-3, gate is 2e-2).  Stages 1/2 must stay bf16: fp8 on
either combine operand or the expert matmul measures 2.4e-2..5.6e-2.

Perf notes (measured on these cores): the PE is power-throttled
(throttle_gpio_2 active ~91%, util limit ~78%) — sustained bf16 cadence is
262 ns per 512-wide matmul vs the 213 ns unthrottled streaming time, with
LDWEIGHTS already hidden by hardware double-buffering.  The kernel is
therefore PE-cycle-bound, not schedule-bound.  Baseline 360 us -> fp8
stage 3 ~329-340 us -> SBUF-resident `we` (B_WE=16, skipping c-half 1's
8 MB re-read that oversubscribed the DMA cap alongside AllGather-0)
measured 322.9 us with stage-1 stalls down from 8.5 to 1.8 us; phase
times 13.5 startup + 138.0 s1 + 34.0 s3 + 134.1 s2 (s2/s3 exactly at
their throttled PE floor) vs a ~315 us floor.  Run-to-run spread is
±6 us (throttle state + cross-core start barrier).  SBUF is at the
208 KB/partition limit — any pool growth must be paid for elsewhere.
"""

import os
import numpy as np
import ml_dtypes

E, C, D_IN, D_OUT = 8, 1024, 4096, 1024
B, S = 2, 2048
TOK = B * S                 # 4096 tokens
N_CORES = 8
S_LOC = TOK // N_CORES      # 512 tokens per core
CH = C // 2                 # c-half = 512
BF16 = ml_dtypes.bfloat16

MODE = os.environ.get("TRN_KERNEL_MODE", "bf16")
LDW_OPT = os.environ.get("TRN_LDW_OPT", "0") == "1"
SKIP_LDW = os.environ.get("TRN_SKIP_LDW", "1") == "1"
NO_MOVE_WAITS = os.environ.get("TRN_NO_MOVE_WAITS", "0") == "1"
# stage-3 residual matmul in fp8 DoubleRow (2 k-tiles/instr, 2x PE rate).
# The residual contribution is ~50x smaller than the combine term, so fp8
# quantization there is invisible (simulated rel-l2 3.35e-3 vs 3.25e-3).
S3FP8 = os.environ.get("TRN_S3FP8", "1") == "1"
F8 = ml_dtypes.float8_e4m3fn

_prog_cache = {}


def _patch_ldw_opt():
    """Compile this kernel's NEFF with walrus' LDWEIGHTS double-buffering
    (--enable-ldw-opt=true): hides the per-matmul 128-column weight load
    behind the previous matmul (~50 ns/MM here). Wrapped so only our
    compile is affected."""
    from concourse import bass_utils
    if getattr(bass_utils, "_ldw_opt_patched", False):
        return
    orig = bass_utils.run_command

    def patched(argv, **kw):
        argv = ["--enable-ldw-opt=true" if a == "--enable-ldw-opt=false" else a
                for a in argv]
        return orig(argv, **kw)

    bass_utils.run_command = patched
    bass_utils._ldw_opt_patched = True


def _build(mode, ldw_opt):
    import concourse.bass as bass  # noqa: F401
    import concourse.mybir as mybir
    from concourse import bacc
    from concourse.tile import TileContext, add_dep_helper

    dt = mybir.dt
    # fp32r must be the declared dtype end-to-end (the BIR verifier rejects
    # fp32-typed producers feeding fp32r matmuls), not a bitcast at the matmul
    io_dt = {"bf16": dt.bfloat16, "fp32r": dt.float32r, "fp32": dt.float32}[mode]
    mm_cast = lambda ap: ap

    nc = bacc.Bacc("TRN2", target_bir_lowering=False, debug=False, num_devices=N_CORES)
    if NO_MOVE_WAITS:
        # keep semaphore waits on the matmuls (not the ldweights) so walrus'
        # --enable-ldw-opt pass accepts the program; instance-level no-op
        nc.move_matmul_waits_to_ldweights = lambda: None

    s3_dt = dt.float8e4 if S3FP8 else io_dt
    fT = nc.declare_dram_parameter("fT", [D_IN, C], io_dt, isOutput=False)
    we = nc.declare_dram_parameter("we", [D_IN, D_OUT], io_dt, isOutput=False)
    cwT = nc.declare_dram_parameter("cwT", [E * C, S_LOC], io_dt, isOutput=False)
    riT = nc.declare_dram_parameter("riT", [D_IN, S_LOC], s3_dt, isOutput=False)
    rw = nc.declare_dram_parameter("rw", [D_IN, D_OUT], s3_dt, isOutput=False)
    out = nc.declare_dram_parameter("out", [S_LOC, D_OUT], dt.float32, isOutput=True)

    # variant tag in a tensor name so differently-compiled builds never share
    # a jax compile-cache entry
    nc.dram_tensor(f"variant_{mode}_{int(ldw_opt)}_{int(SKIP_LDW)}_{int(NO_MOVE_WAITS)}"
                   f"_{int(S3FP8)}", [1, 1], dt.float32)

    ag_in = [nc.dram_tensor(f"ag_in{h}", [CH, D_OUT], io_dt) for h in range(2)]
    ag_out = [nc.dram_tensor(f"ag_out{h}", [N_CORES * CH, D_OUT], io_dt,
                             addr_space="Shared") for h in range(2)]

    KT = D_IN // 128            # 32 contraction tiles
    SUB = 4                     # k-subtiles per DMA'd block
    NBLK = KT // SUB            # 8 blocks
    ECT = (E * C) // 128        # 64 combine contraction tiles
    NFREE = 512                 # ISA cap: s3d3_mm_num_elements <= 512
    NJ = D_OUT // NFREE
    rearr = lambda a: a.rearrange("(n p) d -> p n d", p=128)

    S1SUB = 2                   # finer stage-1 blocks: deeper prefetch pipeline
    S1BLK = KT // S1SUB         # 16 blocks

    bf16_mode = io_dt == dt.bfloat16
    B_FT, B_WE, B_RI, B_RW, B_CW, B_EOAG, B_EO = (
        (12, 9, 3, 3, 3, 6, 2) if bf16_mode else (4, 4, 2, 2, 2, 2, 1))
    if S3FP8 and bf16_mode:
        # fp8 ri/rw tiles are half-size; make them fully resident (8 blocks
        # each) so stage 3 runs with zero DMA waits — its loads prefetch
        # during stage 1, before AllGather chunk 1 saturates the DMA rings
        # (measured 9 us stage-3 stall with 3 bufs).  B_WE=16 makes the
        # expert weights fully resident so c-half 1 skips the 8 MB re-read.
        B_FT, B_WE, B_RI, B_RW, B_EOAG = 9, 16, 8, 8, 4
    with TileContext(nc) as tc:
        with tc.tile_pool(name="p_ft", bufs=B_FT) as p_ft, \
             tc.tile_pool(name="p_we", bufs=B_WE) as p_we, \
             tc.tile_pool(name="p_ri", bufs=B_RI) as p_ri, \
             tc.tile_pool(name="p_rw", bufs=B_RW) as p_rw, \
             tc.tile_pool(name="p_cw", bufs=B_CW) as p_cw, \
             tc.tile_pool(name="p_eoag", bufs=B_EOAG) as p_eoag, \
             tc.tile_pool(name="p_eo", bufs=B_EO) as p_eo, \
             tc.tile_pool(name="p_out", bufs=1) as p_out, \
             tc.tile_pool(name="psum", bufs=1, space="PSUM") as p_ps:


            def mm_pair(psrow, lhsT_ap, rhs_of_j, start, stop):
                """Two matmuls sharing one stationary operand: the second
                skips its LDWEIGHTS (identical weights already in the array)
                and is order-pinned right after the first."""
                prev = None
                for j in range(NJ):
                    m = nc.tensor.matmul(psrow[j], lhsT_ap, rhs_of_j(j),
                                         start=start, stop=stop)
                    # fp32's two-pass matmul requires self-loading weights
                    if j > 0 and SKIP_LDW and bf16_mode:
                        m.ins.ldweights = False
                        add_dep_helper(m.ins, prev.ins, False, "weight-reuse pair order")
                    prev = m
                return prev

            def psum_tiles(tagp):
                return [[p_ps.tile([128, NFREE], dt.float32,
                                   name=f"{tagp}_{i}_{j}", tag=f"ps_{i}_{j}")
                         for j in range(NJ)] for i in range(4)]

            # Stage-3 fp8 operands are fully resident (8 bufs each): their
            # dma_starts sit at the stage-3 program position on sync/scalar,
            # which self-paces them in queue-FIFO order behind all stage-1
            # ft/we loads.  (Every attempt to move or pace them differently —
            # SWDGE burst, SWDGE paced, FIFO-interleaved with stage-1 loads —
            # measured 6-10 us WORSE by starving stage-1's just-in-time feed.)
            rearr2 = lambda a: a.rearrange("(s t p) d -> p s t d", p=128, t=2)

            # ------------- Stage 1: eo_e = fT.T @ we, by c-halves ------------
            # we (the expert weight matrix) is read by BOTH c-halves.  When
            # B_WE >= S1BLK it is loaded once and kept SBUF-resident, cutting
            # stage-1 DMA from 24 MB to 16 MB — the t=40..130 us window runs
            # at the DMA bandwidth cap, and half-1's 8 MB we re-read was a
            # direct contributor to stage-1's just-in-time feed stalls.
            we_resident = B_WE >= S1BLK
            we_tiles = {}
            last_we_dma = None
            for ch in range(2):
                c0 = ch * CH
                psums = psum_tiles(f"s1h{ch}")
                for blk in range(S1BLK):
                    r0 = blk * 128 * S1SUB
                    # only this c-half's columns of fT are needed
                    ft_t = p_ft.tile([128, S1SUB, CH], io_dt, tag="ft", name=f"ft_{ch}_{blk}")
                    load_we = not (we_resident and ch == 1)
                    if load_we:
                        we_t = p_we.tile([128, S1SUB, D_OUT], io_dt, tag="we",
                                         name=f"we_{ch}_{blk}")
                        we_tiles[blk] = we_t
                    else:
                        we_t = we_tiles[blk]
                    if ch == 0 and blk == 0:
                        # per-sub loads: the first matmul starts after 256 KiB,
                        # not after the whole block.  (Splitting these finer
                        # — per-256-col pieces — measured WORSE: the extra
                        # trigger instructions at the queue head delay every
                        # subsequent transfer.)
                        for sub in range(S1SUB):
                            nc.sync.dma_start(
                                out=ft_t[:, sub:sub + 1, :],
                                in_=rearr(fT[r0 + sub * 128:r0 + (sub + 1) * 128, c0:c0 + CH]))
                            last_we_dma = nc.scalar.dma_start(
                                out=we_t[:, sub:sub + 1, :],
                                in_=rearr(we[r0 + sub * 128:r0 + (sub + 1) * 128, :]))
                    else:
                        nc.sync.dma_start(
                            out=ft_t, in_=rearr(fT[r0:r0 + 128 * S1SUB, c0:c0 + CH]))
                        if load_we:
                            # scalar queue: second HWDGE ring, parallel with sync's
                            last_we_dma = nc.scalar.dma_start(
                                out=we_t, in_=rearr(we[r0:r0 + 128 * S1SUB, :]))
                    for sub in range(S1SUB):
                        kt = blk * S1SUB + sub
                        for i in range(4):
                            mm_pair(psums[i],
                                    mm_cast(ft_t[:, sub, i * 128:(i + 1) * 128]),
                                    lambda j, sub=sub: mm_cast(we_t[:, sub, j * NFREE:(j + 1) * NFREE]),
                                    start=(kt == 0), stop=(kt == KT - 1))

                eo_half = p_eo.tile([128, 4, D_OUT], io_dt, tag="eo", name=f"eo_{ch}")
                for i in range(4):
                    for j in range(NJ):
                        nc.vector.tensor_copy(out=eo_half[:, i, j * NFREE:(j + 1) * NFREE],
                                              in_=psums[i][j])
                # gpsimd (SWDGE) queue: keeps this late-gated write out of the
                # HWDGE FIFOs so it can't head-of-line block operand loads
                nc.gpsimd.dma_start(out=rearr(ag_in[ch][:]), in_=eo_half)
                # chunked AllGather: starts while the PE grinds the next phase
                nc.gpsimd.collective_compute(
                    "AllGather", mybir.AluOpType.bypass,
                    replica_groups=[list(range(N_CORES))],
                    ins=[ag_in[ch][:].opt()], outs=[ag_out[ch][:].opt()])

            # ------------- Stage 3: resid partial (w1 folded) ----------------
            psums = psum_tiles("s23")
            last_rw_dma = None
            if S3FP8:
                # fp8 DoubleRow: each matmul contracts TWO 128-k tiles
                # (lhsT [128,2,128], rhs [128,2,256] -> psum [128,256]),
                # halving the PE-active cycles of this stage.
                for blk in range(NBLK):
                    ri_t = p_ri.tile([128, 2, 2, S_LOC], s3_dt, tag="ri",
                                     name=f"ri_{blk}")
                    nc.sync.dma_start(
                        out=ri_t, in_=rearr2(riT[blk * 512:(blk + 1) * 512, :]))
                    rw_t = p_rw.tile([128, 2, 2, D_OUT], s3_dt, tag="rw",
                                     name=f"rw_{blk}")
                    # track for the eoag ordering chain: eoag loads must not
                    # hoist ahead of these on the scalar ring
                    last_rw_dma = nc.scalar.dma_start(
                        out=rw_t, in_=rearr2(rw[blk * 512:(blk + 1) * 512, :]))
                    for sub in range(2):
                        first = blk == 0 and sub == 0
                        for i in range(4):
                            for n in range(4):
                                nc.tensor.matmul(
                                    psums[i][n // 2][:, (n % 2) * 256:(n % 2) * 256 + 256],
                                    ri_t[:, sub, :, i * 128:(i + 1) * 128],
                                    rw_t[:, sub, :, n * 256:(n + 1) * 256],
                                    start=first, stop=False,
                                    perf_mode=mybir.MatmulPerfMode.DoubleRow,
                                    skip_group_check=True)
            else:
                for blk in range(NBLK):
                    ri_t = p_ri.tile([128, SUB, S_LOC], io_dt, tag="ri", name=f"ri_{blk}")
                    nc.sync.dma_start(out=ri_t, in_=rearr(riT[blk * 512:(blk + 1) * 512, :]))
                    rw_t = p_rw.tile([128, SUB, D_OUT], io_dt, tag="rw", name=f"rw_{blk}")
                    last_rw_dma = nc.scalar.dma_start(
                        out=rw_t, in_=rearr(rw[blk * 512:(blk + 1) * 512, :]))
                    for sub in range(SUB):
                        kt = blk * SUB + sub
                        for i in range(4):
                            mm_pair(psums[i],
                                    mm_cast(ri_t[:, sub, i * 128:(i + 1) * 128]),
                                    lambda j, sub=sub: mm_cast(rw_t[:, sub, j * NFREE:(j + 1) * NFREE]),
                                    start=(kt == 0), stop=False)

            # ------------- Stage 2: combine partial (w0 folded) --------------
            out_sb = p_out.tile([128, 4, D_OUT], dt.float32)
            prev_eoag_dma = None
            for blk in range(ECT // SUB):
                half = blk // 8              # ag chunk this block reads
                r0 = (blk % 8) * 512
                cw_t = p_cw.tile([128, SUB, S_LOC], io_dt, tag="cw", name=f"cw_{blk}")
                nc.sync.dma_start(out=cw_t, in_=rearr(cwT[blk * 512:(blk + 1) * 512, :]))
                eo_t = p_eoag.tile([128, SUB, D_OUT], io_dt, tag="eoag", name=f"eoag_{blk}")
                eoag_dma = nc.scalar.dma_start(out=eo_t, in_=rearr(ag_out[half][r0:r0 + 512, :]))
                # Scalar-queue order: eoag loads wait on the AllGathers, so pin
                # them after every stage-1/3 operand load and in block order —
                # otherwise the scheduler can hoist one ahead and head-of-line
                # block the HWDGE FIFO on the collective (measured 44 us stall).
                prev = prev_eoag_dma if prev_eoag_dma is not None else (
                    last_rw_dma if last_rw_dma is not None else last_we_dma)
                if prev is not None:
                    add_dep_helper(eoag_dma.ins, prev.ins, False,
                                   "eoag after stage-1/3 loads, in block order")
                prev_eoag_dma = eoag_dma
                last_blk = blk == ECT // SUB - 1
                if not last_blk:
                    for sub in range(SUB):
                        for i in range(4):
                            mm_pair(psums[i],
                                    mm_cast(cw_t[:, sub, i * 128:(i + 1) * 128]),
                                    lambda j, sub=sub: mm_cast(eo_t[:, sub, j * NFREE:(j + 1) * NFREE]),
                                    start=False, stop=False)
                else:
                    # last block: finish groups one at a time so the PSUM->SBUF
                    # copies and output DMAs overlap the remaining matmuls
                    for i in range(4):
                        for j in range(NJ):
                            for sub in range(SUB):
                                nc.tensor.matmul(
                                    psums[i][j],
                                    mm_cast(cw_t[:, sub, i * 128:(i + 1) * 128]),
                                    mm_cast(eo_t[:, sub, j * NFREE:(j + 1) * NFREE]),
                                    start=False, stop=(sub == SUB - 1))
                            nc.vector.tensor_copy(
                                out=out_sb[:, i, j * NFREE:(j + 1) * NFREE],
                                in_=psums[i][j])
                        nc.sync.dma_start(
                            out=out[i * 128:(i + 1) * 128, :].rearrange("(n p) d -> p n d", p=128),
                            in_=out_sb[:, i:i + 1, :])

    nc.finalize()
    return nc


def _get_prog(mode, ldw_opt):
    key = (mode, ldw_opt)
    if key not in _prog_cache:
        if ldw_opt:
            _patch_ldw_opt()
        _prog_cache[key] = _build(mode, ldw_opt)
    return _prog_cache[key]


def _prep_in_maps(inputs, expert_w, residual_w, combine_weights, residual_weight, mode):
    np_dt = BF16 if mode == "bf16" else np.float32
    s3_dt = F8 if S3FP8 else np_dt
    front = inputs[:E * C].reshape(E, C, D_IN)
    resid = inputs[E * C:]                       # [TOK, D_IN]
    rwt = residual_weight.reshape(TOK, 2)
    w0, w1 = rwt[:, 0], rwt[:, 1]

    rw_cast = np.ascontiguousarray(residual_w.astype(s3_dt))
    resid_s = resid * w1[:, None]                # fold w1 (fp32)
    in_maps = []
    for r in range(N_CORES):
        sl = slice(r * S_LOC, (r + 1) * S_LOC)
        fT = np.ascontiguousarray(front[r].T.astype(np_dt))              # [D_IN, C]
        we = np.ascontiguousarray(expert_w[r].astype(np_dt))             # [D_IN, D_OUT]
        cw_s = combine_weights[sl] * w0[sl, None, None]                  # [S_LOC, E, C]
        # contraction rows ordered (c-half chunk, expert, c-within-half) to
        # match the chunked AllGather's concatenation
        cwT = np.ascontiguousarray(
            cw_s.reshape(S_LOC, E, 2, CH).transpose(2, 1, 3, 0).reshape(E * C, S_LOC)
            .astype(np_dt))
        riT = np.ascontiguousarray(resid_s[sl].T.astype(s3_dt))          # [D_IN, S_LOC]
        in_maps.append({"fT": fT, "we": we, "cwT": cwT, "riT": riT, "rw": rw_cast})
    return in_maps


def _run(inputs, expert_w, expert_b, residual_w, residual_b,
         combine_weights, residual_weight, mode=None, ldw_opt=None, trace=False):
    import jax
    try:
        if jax.config.jax_compilation_cache_dir is None:
            jax.config.update("jax_compilation_cache_dir", "/tmp/jax_cache_trn_moe")
            jax.config.update("jax_persistent_cache_min_compile_time_secs", 0.5)
    except Exception:
        pass
    from concourse.bass_utils import run_bass_kernel_spmd

    mode = mode or MODE
    ldw_opt = LDW_OPT if ldw_opt is None else ldw_opt
    inputs = np.asarray(inputs, dtype=np.float32)
    expert_w = np.asarray(expert_w, dtype=np.float32)
    expert_b = np.asarray(expert_b, dtype=np.float32)
    residual_w = np.asarray(residual_w, dtype=np.float32)
    residual_b = np.asarray(residual_b, dtype=np.float32)
    combine_weights = np.asarray(combine_weights, dtype=np.float32)
    residual_weight = np.asarray(residual_weight, dtype=np.float32)

    nc = _get_prog(mode, ldw_opt)
    in_maps = _prep_in_maps(inputs, expert_w, residual_w, combine_weights,
                            residual_weight, mode)
    res = run_bass_kernel_spmd(nc, in_maps, list(range(N_CORES)), trace=trace)
    out = np.concatenate([res.results[r]["out"] for r in range(N_CORES)], axis=0)

    # exact bias contributions (zero in practice, but keep the math honest)
    rwt = residual_weight.reshape(TOK, 2)
    if residual_b.any():
        out = out + rwt[:, 1:2] * residual_b[None, :]
    if expert_b.any():
        cs = combine_weights.sum(axis=2)                    # [TOK, E]
        out = out + rwt[:, 0:1] * (cs @ expert_b)
    return out.reshape(B, S, D_OUT).astype(np.float32), res


def kernel(**kw):
    out, _ = _run(**kw)
    return out

